# revision 20
# baseline (speedup 1.0000x reference)
"""Bass/Trainium2 kernel for BasicRNN: h_t = tanh(x_t @ W_xh + h_{t-1} @ W_hh + b).

Full shapes: inputs [128, 512, 1024] f32, W_xh [1024,1024], W_hh [1024,1024], b [1024].
Output: states [512, 128, 1024] f32 (T, B, U).

Sharding: data-parallel over batch across 8 cores (16 rows/core); weights replicated.

Per-core plan:
  Phase 1: xw = x @ W_xh + b as one big GEMM ([16*512, 1024] x [1024, 1024]).
           x row-chunks are PE-transposed to put the contraction (D) on
           partitions; fp32r (tf32) matmuls at full PE rate; result stored to
           internal DRAM in [T, 16, 1024] layout (contiguous per-step reads).
  Phase 2: 512 serial steps. h kept transposed (hT: [U on partitions, 16]) so
           each step's matmul streams W_hh as the moving operand at N=512
           (full 128-lane utilization). z = hT.T @ W_hh accumulated over 8
           K-tiles per 512-wide half; xw_t added on DVE; tanh on ACT; h_new
           PE-transposed back into hT for the next step.
"""

import sys

sys.path.insert(0, "/opt/trn_rl_repo")

import numpy as np

import concourse.bass as bass
import concourse.mybir as mybir
from concourse import bacc
from concourse.bass import ds, ts
from concourse.masks import make_identity
from concourse.tile import TileContext
from concourse.bass_utils import run_bass_kernel_spmd

F32 = mybir.dt.float32
F32R = mybir.dt.float32r

B_FULL = 128
T_FULL = 512
D = 1024
U = 1024
N_CORES = 8
B_LOC = B_FULL // N_CORES  # 16
KT = D // 128  # 8 contraction tiles
UT = U // 128  # 8 output tiles


def build_rnn(T=T_FULL, n_cores=N_CORES, mm_dtype=F32R):
    """Build the per-core Bass program (SPMD across n_cores)."""
    assert T % 128 == 0
    chunks_per_b = T // 128

    nc = bacc.Bacc("TRN2", target_bir_lowering=False, debug=False,
                   num_devices=n_cores)

    x_dram = nc.dram_tensor("x", [B_LOC, T, D], F32, kind="ExternalInput")
    wxh_dram = nc.dram_tensor("wxh", [D, U], F32, kind="ExternalInput")
    whh_dram = nc.dram_tensor("whh", [U, U], F32, kind="ExternalInput")
    b_dram = nc.dram_tensor("b", [U], F32, kind="ExternalInput")
    out_dram = nc.dram_tensor("out", [T, B_LOC, U], F32, kind="ExternalOutput")
    n_tq = T // 128  # t-quarter scratch tensors so phase-2 deps are fine-grained
    xw_drams = [nc.dram_tensor(f"xw{j}", [128, B_LOC, U], mm_dtype)
                for j in range(n_tq)]

    x_flat = x_dram.ap().rearrange("b t d -> (b t) d")

    with TileContext(nc) as tc:
        with tc.tile_pool(name="persist", bufs=1) as persist:
            whh_sb = persist.tile([128, KT, U], mm_dtype)  # W_hh k-tiles
            wxh_sb = persist.tile([128, KT, U], mm_dtype)  # W_xh k-tiles
            b_row = persist.tile([1, U], F32)
            b_full = persist.tile([128, U], F32)
            ones_row = persist.tile([1, 128], F32)
            hT_lo = persist.tile([128, KT // 2, B_LOC], mm_dtype)
            hT_hi = persist.tile([128, KT // 2, B_LOC], mm_dtype)
            ident = persist.tile([128, 128], F32)
            ident16r = persist.tile([B_LOC, B_LOC], mm_dtype)

            make_identity(nc, ident)
            nc.vector.tensor_copy(out=ident16r, in_=ident[:B_LOC, :B_LOC])
            with tc.tile_pool(name="wstage", bufs=4) as wstage:
                for k in range(KT):
                    for src_dram, dst in ((wxh_dram, wxh_sb), (whh_dram, whh_sb)):
                        stg = wstage.tile([128, U], F32)
                        nc.sync.dma_start(out=stg, in_=src_dram[ts(k, 128), :])
                        nc.vector.tensor_copy(out=dst[:, k, :], in_=stg)
            nc.sync.dma_start(out=b_row, in_=b_dram.ap().unsqueeze(0))
            nc.vector.memset(ones_row, 1.0)

            id16 = ident[:B_LOC, :B_LOC]
            n_chunks = B_LOC * T // 128

            with (
                tc.tile_pool(name="p2_xw", bufs=6) as xwp,
                tc.tile_pool(name="p2_h", bufs=3) as hp,
                tc.tile_pool(name="p2_psZ", bufs=2, space="PSUM") as pZ2,
                tc.tile_pool(name="p2_psT", bufs=1, space="PSUM") as pT2,
                tc.tile_pool(name="p1_x", bufs=3) as xp,
                tc.tile_pool(name="p1_xT", bufs=3) as xtp,
                tc.tile_pool(name="p1_out", bufs=3) as outp,
                tc.tile_pool(name="p1_psT", bufs=1, space="PSUM") as pTp,
                tc.tile_pool(name="p1_psZ", bufs=1, space="PSUM") as pZp,
            ):
                # broadcast b across partitions: b_full = ones.T @ b_row
                for half in range(2):
                    psB = pZp.tile([128, 512], F32, tag="psZ")
                    nc.tensor.matmul(psB, ones_row,
                                     b_row[:, ds(half * 512, 512)],
                                     start=True, stop=True)
                    nc.vector.tensor_copy(out=b_full[:, ds(half * 512, 512)],
                                          in_=psB)

                def emit_chunk(c):
                    b_idx = c // chunks_per_b
                    t0 = (c % chunks_per_b) * 128
                    x_sb = xp.tile([128, D], F32)
                    nc.sync.dma_start(out=x_sb, in_=x_flat[ts(c, 128), :])
                    xT = xtp.tile([128, KT, 128], mm_dtype)
                    for k in range(KT):
                        psT = pTp.tile([128, 128], F32)
                        nc.tensor.transpose(psT, x_sb[:, ts(k, 128)], ident)
                        nc.vector.tensor_copy(out=xT[:, k, :], in_=psT)
                    o_sb = outp.tile([128, U], mm_dtype)
                    for half in range(2):
                        psZ = pZp.tile([128, 512], F32)
                        for k in range(KT):
                            nc.tensor.matmul(
                                psZ,
                                xT[:, k, :],
                                wxh_sb[:, k, ds(half * 512, 512)],
                                start=(k == 0),
                                stop=(k == KT - 1),
                            )
                        nc.vector.tensor_add(
                            out=o_sb[:, ds(half * 512, 512)],
                            in0=psZ,
                            in1=b_full[:, ds(half * 512, 512)],
                        )
                    nc.sync.dma_start(out=xw_drams[t0 // 128][:, b_idx, :],
                                      in_=o_sb)

                state = {"h_prev": None}

                def emit_step(t):
                    h_prev = state["h_prev"]
                    xw_t = xwp.tile([B_LOC, U], mm_dtype)
                    nc.sync.dma_start(out=xw_t,
                                      in_=xw_drams[t // 128][t % 128])
                    h_new = hp.tile([B_LOC, U], F32)
                    psz0 = pZ2.tile([B_LOC, 512], F32)
                    psz1 = pZ2.tile([B_LOC, 512], F32)
                    # xw enters via identity matmul; first write of each group
                    nc.tensor.matmul(psz0, ident16r, xw_t[:, ds(0, 512)],
                                     start=True, stop=(t == 0))
                    nc.tensor.matmul(psz1, ident16r, xw_t[:, ds(512, 512)],
                                     start=True, stop=(t == 0))
                    if t > 0:
                        for k in range(4):
                            nc.tensor.matmul(
                                psz0, hT_lo[:, k, :], whh_sb[:, k, ds(0, 512)],
                                start=False, stop=False)
                        for k in range(4):
                            nc.tensor.matmul(
                                psz1, hT_lo[:, k, :], whh_sb[:, k, ds(512, 512)],
                                start=False, stop=False)
                        # late half of h_{t-1}'s transpose (k-tiles 4-7)
                        psT_hi = pT2.tile([128, KT // 2, B_LOC], F32,
                                          tag="psT_hi")
                        for j in range(4):
                            nc.tensor.transpose(
                                psT_hi[:, j, :],
                                h_prev[:, ts(4 + j, 128)], id16)
                        nc.vector.tensor_copy(out=hT_hi[:, 0:2, :],
                                              in_=psT_hi[:, 0:2, :])
                        nc.vector.tensor_copy(out=hT_hi[:, 2:4, :],
                                              in_=psT_hi[:, 2:4, :])
                        for k in range(4):
                            nc.tensor.matmul(
                                psz0, hT_hi[:, k, :],
                                whh_sb[:, 4 + k, ds(0, 512)],
                                start=False, stop=(k == 3))
                        for k in range(4):
                            nc.tensor.matmul(
                                psz1, hT_hi[:, k, :],
                                whh_sb[:, 4 + k, ds(512, 512)],
                                start=False, stop=(k == 3))
                    nc.scalar.activation(
                        h_new[:, ds(0, 512)], psz0,
                        mybir.ActivationFunctionType.Tanh)
                    nc.scalar.activation(
                        h_new[:, ds(512, 512)], psz1,
                        mybir.ActivationFunctionType.Tanh)
                    if t < T - 1:
                        # early half of h_t's transpose (k-tiles 0-3)
                        psT_lo = pT2.tile([128, KT // 2, B_LOC], F32,
                                          tag="psT_lo")
                        for k in range(4):
                            nc.tensor.transpose(
                                psT_lo[:, k, :], h_new[:, ts(k, 128)], id16)
                        nc.vector.tensor_copy(out=hT_lo[:, 0:2, :],
                                              in_=psT_lo[:, 0:2, :])
                        nc.vector.tensor_copy(out=hT_lo[:, 2:4, :],
                                              in_=psT_lo[:, 2:4, :])
                    nc.sync.dma_start(out=out_dram[t], in_=h_new)
                    state["h_prev"] = h_new

                # interleave: quarter j's chunks, then quarter j's steps;
                # later quarters' chunk work fills phase-2 stall windows
                for j in range(chunks_per_b):
                    for b_i in range(B_LOC):
                        emit_chunk(b_i * chunks_per_b + j)
                    for t in range(128 * j, 128 * (j + 1)):
                        emit_step(t)

    nc.compile()
    return nc


_CACHE = {}


def _get_nc(T, n_cores):
    key = (T, n_cores)
    if key not in _CACHE:
        _CACHE[key] = build_rnn(T, n_cores)
    return _CACHE[key]


class _Runner:
    """Caches the jitted PJRT executable so repeat kernel() calls skip
    recompilation (mirrors bass2jax.run_bass_via_pjrt's multi-core path)."""

    def __init__(self, nc, n_cores):
        import jax
        from jax.sharding import Mesh, PartitionSpec
        from jax.experimental.shard_map import shard_map
        from concourse import bass2jax
        from concourse.bass2jax import _bass_exec_p, partition_id_tensor

        bass2jax.install_neuronx_cc_hook()
        self.jax = jax
        self.n_cores = n_cores
        partition_name = (nc.partition_id_tensor.name
                          if nc.partition_id_tensor else None)
        in_names, out_names, out_avals = [], [], []
        for alloc in nc.m.functions[0].allocations:
            if not isinstance(alloc, mybir.MemoryLocationSet):
                continue
            name = alloc.memorylocations[0].name
            if alloc.kind == "ExternalInput":
                if name != partition_name:
                    in_names.append(name)
            elif alloc.kind == "ExternalOutput":
                out_names.append(name)
                out_avals.append(jax.core.ShapedArray(
                    tuple(alloc.tensor_shape), mybir.dt.np(alloc.dtype)))
        self.in_names = in_names
        self.out_names = out_names
        self.out_avals = out_avals
        n_params = len(in_names)
        all_names = in_names + out_names
        if partition_name is not None:
            all_names.append(partition_name)
        donate = tuple(range(n_params, n_params + len(out_avals)))

        def _body(*args):
            operands = list(args)
            if partition_name is not None:
                operands.append(partition_id_tensor())
            return tuple(_bass_exec_p.bind(
                *operands,
                out_avals=tuple(out_avals),
                in_names=tuple(all_names),
                out_names=tuple(out_names),
                lowering_input_output_aliases=(),
                sim_require_finite=True,
                sim_require_nnan=True,
                nc=nc,
            ))

        devices = jax.devices()[:n_cores]
        self.mesh = Mesh(np.asarray(devices), ("core",))
        self.sharding = jax.sharding.NamedSharding(
            self.mesh, PartitionSpec("core"))
        self.fn = jax.jit(
            shard_map(_body, mesh=self.mesh,
                      in_specs=(PartitionSpec("core"),) * (n_params + len(out_avals)),
                      out_specs=(PartitionSpec("core"),) * len(out_avals),
                      check_rep=False),
            donate_argnums=donate, keep_unused=True,
        )

    def __call__(self, in_maps):
        jax = self.jax
        import jax.numpy as jnp
        concat_in = [
            jax.device_put(
                np.concatenate([np.asarray(m[name]) for m in in_maps], axis=0),
                self.sharding)
            for name in self.in_names
        ]
        bufs = [
            jax.device_put(
                jnp.zeros((self.n_cores * a.shape[0], *a.shape[1:]), a.dtype),
                self.sharding)
            for a in self.out_avals
        ]
        outs = self.fn(*concat_in, *bufs)
        outs = [np.asarray(o) for o in outs]
        return [
            {name: outs[i].reshape(self.n_cores, *self.out_avals[i].shape)[c]
             for i, name in enumerate(self.out_names)}
            for c in range(self.n_cores)
        ]


_RUNNERS = {}


def run(inputs, W_xh, W_hh, b, T=T_FULL, n_cores=N_CORES):
    nc = _get_nc(T, n_cores)
    inputs = np.ascontiguousarray(inputs, dtype=np.float32)
    W_xh = np.ascontiguousarray(W_xh, dtype=np.float32)
    W_hh = np.ascontiguousarray(W_hh, dtype=np.float32)
    b = np.ascontiguousarray(b, dtype=np.float32)
    in_maps = [
        {
            "x": inputs[c * B_LOC:(c + 1) * B_LOC],
            "wxh": W_xh,
            "whh": W_hh,
            "b": b,
        }
        for c in range(n_cores)
    ]
    key = (T, n_cores)
    try:
        if key not in _RUNNERS:
            _RUNNERS[key] = _Runner(nc, n_cores)
        results = _RUNNERS[key](in_maps)
    except Exception:
        _RUNNERS.pop(key, None)
        results = run_bass_kernel_spmd(nc, in_maps, list(range(n_cores))).results
    out = np.empty((T, n_cores * B_LOC, U), dtype=np.float32)
    for c in range(n_cores):
        out[:, c * B_LOC:(c + 1) * B_LOC, :] = results[c]["out"]
    return out


def kernel(inputs, W_xh, W_hh, b):
    return run(inputs, W_xh, W_hh, b)
